# revision 39
# baseline (speedup 1.0000x reference)
"""HeightAwarePointNetTiny on 8 Trainium2 NeuronCores (Bass/Tile), v2.

Same math as v1 (see kernel.py docstring): each LocalAggBlock is
    out_i = relu(v_i + max_{j in KNN(i)} u_j)
with u/v from split weights, KNN top-16 via PE score rows + DVE
max8/max_index/match_replace, neighbor gathers via GPSIMD ap_gather.

v2 changes target the per-call host/tunnel overhead that dominates
wall-clock: the 27 per-core input tensors (~11 MB on the wire, weights
replicated x8) are repacked into three —
  xp  f32 [5, 8192]  rows 0-3 = cloud x^T, row 4 = fp32 params (biases,
                     height-mixer matrix, sigmoid params)
  qp  f32 [4, 4096]  this core's query half of x^T
  wsh f16 [1, 36864] 1/8th shard of the fp16 weight pack; the full pack
                     is reassembled on-device with an 8-way AllGather
(~1.9 MB total).  Weights are fp16 on the wire only; they are upcast to
fp32 in SBUF before any matmul.  Device-side the per-tile wrap_all
replication DMAs (8 per tile) are batched into 8 total, per-tile output
and hag DMAs are hoisted, and the constant score rows are memset instead
of DMA-copied.

Sharding: core c owns cloud c//2, query half c%2 (4096 rows).  Cross-core
data: weight shards (AllGather over all 8), f1 (AllGather over pairs),
global max pool (AllReduce-max over pairs).
"""
import sys
sys.path.insert(0, '/opt/trn_rl_repo')
import numpy as np
from contextlib import ExitStack

try:
    # Persistent XLA compilation cache: run_bass_kernel_spmd re-jits a fresh
    # closure every call, so without this each kernel() invocation pays a
    # full NEFF recompile (~0.6 s) even though the program is unchanged.
    import jax
    jax.config.update("jax_compilation_cache_dir", "/tmp/jaxcache")
    jax.config.update("jax_persistent_cache_min_entry_size_bytes", 0)
    jax.config.update("jax_persistent_cache_min_compile_time_secs", 0.0)
except Exception:
    pass

import concourse.bass as bass
import concourse.tile as tile
from concourse import bacc, mybir

dt = mybir.dt
F32 = dt.float32
F16 = dt.float16

B, N, IN_CH = 4, 8192, 4
K = 16
W0, W1, W2 = 64, 128, 256
NUM_CLASSES = 3
NCORES = 8
P = 128
CH = 512
HALVES = 2
WSHARD = 36864                      # fp16 elems per core shard
WTOT = NCORES * WSHARD

# flat fp16 pack offsets (within the full 8*WSHARD pack)
_S = WSHARD
WOFF = {
    "w2_u_a": (0 * _S, 128, 256),
    "w1_u_a0": (0 * _S + 32768, 32, 128),
    "w2_v_a": (1 * _S, 128, 256),
    "w1_u_a1": (1 * _S + 32768, 32, 128),
    "glob_k0": (2 * _S, 128, 256),
    "w1_v_a0": (2 * _S + 32768, 32, 128),
    "glob_k1": (3 * _S, 128, 256),
    "w1_v_a1": (3 * _S + 32768, 32, 128),
    "h1a_k0": (4 * _S, 128, 256),
    "w2_u_b": (4 * _S + 32768, 3, 256),
    "w2_v_b": (4 * _S + 33536, 3, 256),
    "w1_u_b": (4 * _S + 34304, 3, 128),
    "w1_v_b": (4 * _S + 34688, 3, 128),
    "stem_w": (4 * _S + 35072, 4, 64),
    "h2_k0": (4 * _S + 35328, 128, 3),
    "h2_k1": (4 * _S + 35712, 128, 3),
    "h1a_k1": (5 * _S, 128, 256),
    "h1g_k0": (6 * _S, 128, 256),
    "h1g_k1": (7 * _S, 128, 256),
}
# fp32 params layout in xp row 4
PB_BT = 0           # Bt [128, 8]: cols stem_b(64), b1_b, b2_b0, b2_b1,
                    #              glob_b0, glob_b1, h1_b0, h1_b1
PB_MC4 = 1024       # mc4 [4, 4]: cols 0-2 height-mixer, col 3 rows 0-2 cvec
PB_H2B = 1040       # h2_b [3, 1]
PB_SIG = 1043       # sig_par [1, 3]


def build_program(n=N, ncores=NCORES):
    nq = n // HALVES
    nt = nq // P
    nch = n // CH
    nqch = nq // CH
    nc = bacc.Bacc("TRN2", target_bir_lowering=False, debug=False,
                   num_devices=ncores)

    xp = nc.dram_tensor("xp", [5, n], F32, kind="ExternalInput")
    qp = nc.dram_tensor("qp", [4, nq], F32, kind="ExternalInput")
    wsh = nc.dram_tensor("wsh", [1, WSHARD], F16, kind="ExternalInput")

    out_lg = nc.dram_tensor("out_lg", [NUM_CLASSES, nq], F16,
                            kind="ExternalOutput")
    wloc = nc.dram_tensor("wloc", [1, WSHARD], F16)
    wfull = nc.dram_tensor("wfull", [NCORES, WSHARD], F16)
    coords_dram = nc.dram_tensor("coords_dram", [3, n], F32)
    xx_dram = nc.dram_tensor("xx_dram", [1, n], F32)
    xxq_dram = nc.dram_tensor("xxq_dram", [1, nq], F32)
    f1_loc = nc.dram_tensor("f1_loc", [W1, nq], F32)
    f1_gath = nc.dram_tensor("f1_gath", [HALVES, W1, nq], F32)
    g_loc = nc.dram_tensor("g_loc", [W2, 1], F32)
    g_red = nc.dram_tensor("g_red", [W2, 1], F32)
    PAIRS = [[c, c + 1] for c in range(0, ncores, 2)] if ncores > 1 else []
    ALL8 = [list(range(ncores))] if ncores > 1 else []

    Relu = mybir.ActivationFunctionType.Relu
    Copy = mybir.ActivationFunctionType.Copy
    Sigmoid = mybir.ActivationFunctionType.Sigmoid
    Square = mybir.ActivationFunctionType.Square
    AX = mybir.AxisListType.X
    MAX = mybir.AluOpType.max
    ADD = mybir.AluOpType.add

    with tile.TileContext(nc) as tc, ExitStack() as ctx:
        pers = ctx.enter_context(tc.tile_pool(name="pers", bufs=1))
        lpool = ctx.enter_context(tc.tile_pool(name="lp", bufs=2))
        stg = ctx.enter_context(tc.tile_pool(name="stg", bufs=2))
        ppool = ctx.enter_context(tc.tile_pool(name="ps", bufs=4, space="PSUM"))

        def mm_chain(dst, dst_sl, parts, act=Copy, bias=0.0, scale=1.0,
                     shape=(P, CH)):
            ps = ppool.tile(list(shape), F32, tag="mm", name="mmps")
            for ix, (lhsT, rhs) in enumerate(parts):
                nc.tensor.matmul(ps[:], lhsT, rhs, start=(ix == 0),
                                 stop=(ix == len(parts) - 1))
            nc.scalar.activation(dst[:, dst_sl], ps[:], act, bias=bias,
                                 scale=scale)

        # ---- weight shard exchange ----
        nc.sync.dma_start(wloc.ap(), wsh.ap())
        if ALL8:
            nc.gpsimd.collective_compute(
                "AllGather", mybir.AluOpType.bypass, replica_groups=ALL8,
                ins=[wloc.ap()], outs=[wfull.ap()])
        else:
            for r in range(NCORES):
                nc.sync.dma_start(wfull.ap()[r], wloc.ap())

        # ---- unpack fp32 params from xp row 4 ----
        def par_tile(tag, p_, f_, off):
            t32 = pers.tile([p_, f_], F32, tag=tag, name=tag)
            nc.sync.dma_start(
                t32[:], xp.ap()[4, off:off + p_ * f_].rearrange(
                    "(p f) -> p f", p=p_))
            return t32

        Bt = par_tile("Bt", P, 8, PB_BT)
        mc4 = par_tile("mc4", 4, 4, PB_MC4)
        h2b = par_tile("h2b", NUM_CLASSES, 1, PB_H2B)
        sig = par_tile("sig", 1, 3, PB_SIG)

        # ---- unpack fp16 weights -> fp32 SBUF tiles ----
        W = {}
        for name, (off, p_, f_) in WOFF.items():
            if name[-1] in "01" and name[:-1] in ("w1_u_a", "w1_v_a"):
                continue
            s_, o_ = off // WSHARD, off % WSHARD
            t16 = stg.tile([p_, f_], F16, tag="w16stg", name=f"s_{name}")
            nc.sync.dma_start(
                t16[:], wfull.ap()[s_, o_:o_ + p_ * f_].rearrange(
                    "(p f) -> p f", p=p_))
            t32 = pers.tile([p_, f_], F32, tag=f"W_{name}", name=name)
            nc.scalar.activation(t32[:], t16[:], Copy)
            W[name] = t32
        # split [32, W1] halves: DMA both into one fp16 tile, convert once
        for base in ("w1_u_a", "w1_v_a"):
            t16 = stg.tile([W0, W1], F16, tag="w16stg", name=f"s_{base}")
            for half in (0, 1):
                off, p_, f_ = WOFF[f"{base}{half}"]
                s_, o_ = off // WSHARD, off % WSHARD
                nc.sync.dma_start(
                    t16[32 * half:32 * half + 32, :],
                    wfull.ap()[s_, o_:o_ + p_ * f_].rearrange(
                        "(p f) -> p f", p=p_))
            t32 = pers.tile([W0, W1], F32, tag=f"W_{base}", name=base)
            nc.scalar.activation(t32[:], t16[:], Copy)
            W[base] = t32
        w1_u_a = W["w1_u_a"]
        w1_v_a = W["w1_v_a"]

        stem_b = Bt[0:W0, 0:1]
        b1_b = Bt[:, 1:2]
        b2_b = Bt[:, 2:4]
        glob_b = Bt[:, 4:6]
        h1_b = Bt[:, 6:8]
        cvec = mc4[0:3, 3:4]

        wrap_all = pers.tile([P, nt * P], dt.uint16, tag="wrap_all")
        ones3 = pers.tile([3, 1], F32, tag="ones3")
        nc.vector.memset(ones3[:], 1.0)

        with tc.tile_pool(name="poolC", bufs=1) as poolC:
            q5 = poolC.tile([5, nq], F32, tag="q5")
            f1T = poolC.tile([W1, nq], F32, tag="f1T")


            with tc.tile_pool(name="poolB", bufs=1) as poolB:
                rhs5 = poolB.tile([5, n], F32, tag="rhs5")
                U1T = poolB.tile([W1, n], F32, tag="U1T")
                giT_all = poolB.tile([32, nt * P], dt.uint16, tag="giT_all")
                # rows 0-2 and 4 are overwritten below; row 3 keeps -1
                nc.vector.memset(rhs5[:], -1.0)
                # rows 0-3 are overwritten below; row 4 keeps +1
                nc.vector.memset(q5[:], 1.0)

                # ---- streamed setup over candidate chunks ----
                def cand_chunk(sl, src):
                    xch = stg.tile([4, CH], F32, tag="xch", name="xch")
                    nc.sync.dma_start(xch[:], src)
                    cch = stg.tile([3, CH], F32, tag="cch", name="cch")
                    ps = ppool.tile([3, CH], F32, tag="mm", name="csps")
                    nc.tensor.matmul(ps[:], mc4[0:4, 0:3], xch[:],
                                     start=True, stop=True)
                    nc.vector.tensor_scalar(cch[:], ps[:], cvec, None,
                                            op0=ADD)
                    nc.scalar.activation(rhs5[0:3, sl], cch[:], Copy,
                                         scale=2.0)
                    nc.sync.dma_start(coords_dram.ap()[:, sl], cch[:])
                    sqs = stg.tile([3, CH], F32, tag="sqs", name="sqs")
                    nc.scalar.activation(sqs[:], cch[:], Square)
                    psx = ppool.tile([1, CH], F32, tag="mm", name="xxps")
                    nc.tensor.matmul(psx[:], ones3[:], sqs[:],
                                     start=True, stop=True)
                    xxs = stg.tile([1, CH], F32, tag="xxs", name="xxs")
                    nc.scalar.activation(xxs[:], psx[:], Copy, scale=-1.0)
                    nc.sync.dma_start(xx_dram.ap()[:, sl], xxs[:])
                    f64 = stg.tile([W0, CH], F32, tag="f64", name="f64")
                    mm_chain(f64, slice(0, CH),
                             [(W["stem_w"][:], xch[:])],
                             act=Relu, bias=stem_b, shape=(W0, CH))
                    mm_chain(U1T, sl, [(w1_u_a[:], f64[:]),
                                       (W["w1_u_b"][:], cch[:])])

                with tc.For_i(0, nch, 2) as ic:
                    for u in range(2):
                        sl = bass.ds(ic * CH + u * CH, CH)
                        cand_chunk(sl, xp.ap()[0:4, sl])
                nc.sync.dma_start(rhs5[4:5, :], xx_dram.ap())

                # ---- streamed setup over query chunks (V1 -> f1T) ----
                def query_chunk(sl):
                    xch = stg.tile([4, CH], F32, tag="xch", name="xch")
                    nc.sync.dma_start(xch[:], qp.ap()[:, sl])
                    ps = ppool.tile([3, CH], F32, tag="mm", name="qcps")
                    nc.tensor.matmul(ps[:], mc4[0:4, 0:3], xch[:],
                                     start=True, stop=True)
                    nc.vector.tensor_scalar(q5[0:3, sl], ps[:], cvec, None,
                                            op0=ADD)
                    sqs = stg.tile([3, CH], F32, tag="sqs", name="sqs")
                    nc.scalar.activation(sqs[:], q5[0:3, sl], Square)
                    psx = ppool.tile([1, CH], F32, tag="mm", name="xxpsq")
                    nc.tensor.matmul(psx[:], ones3[:], sqs[:],
                                     start=True, stop=True)
                    xxs = stg.tile([1, CH], F32, tag="xxs", name="xxs")
                    nc.scalar.activation(xxs[:], psx[:], Copy)
                    nc.sync.dma_start(xxq_dram.ap()[:, sl], xxs[:])
                    f64 = stg.tile([W0, CH], F32, tag="f64", name="f64")
                    mm_chain(f64, slice(0, CH),
                             [(W["stem_w"][:], xch[:])],
                             act=Relu, bias=stem_b, shape=(W0, CH))
                    mm_chain(f1T, sl, [(w1_v_a[:], f64[:]),
                                       (W["w1_v_b"][:], q5[0:3, sl])])

                with tc.For_i(0, nqch, 2) as iq:
                    for u in range(2):
                        query_chunk(bass.ds(iq * CH + u * CH, CH))
                nc.sync.dma_start(q5[3:4, :], xxq_dram.ap())

                # ---- selection: srow -> exact top-16 -> giT_all ----
                with tc.tile_pool(name="spool", bufs=1) as spool:
                    srow = spool.tile([P, n], F32, tag="srow", name="srow")
                    w16 = lpool.tile([P, K], F32, tag="w16", name="w16")
                    gi = lpool.tile([P, K], dt.uint16, tag="gi", name="gi")
                    gip = lpool.tile([P, 32], dt.uint16, tag="gip",
                                     name="gip")
                    qt5 = spool.tile([5, P], F32, tag="qt5", name="qt5")
                    nc.vector.memset(gip[:], 0)
                    with tc.For_i(0, nt) as t:
                        tsl = bass.ts(t, P)
                        nc.scalar.activation(qt5[:], q5[:, tsl], Copy)
                        for i in range(nch):
                            ps = ppool.tile([P, CH], F32, tag="mm",
                                            name="sps")
                            nc.tensor.matmul(ps[:], qt5[:],
                                             rhs5[:, bass.ts(i, CH)],
                                             start=True, stop=True)
                            nc.scalar.activation(srow[:, bass.ts(i, CH)],
                                                 ps[:], Copy, scale=1.0)
                        nc.vector.max(w16[:, 0:8], srow[:])
                        nc.vector.max_index(gi[:, 0:8], w16[:, 0:8],
                                            srow[:])
                        nc.vector.match_replace(srow[:], w16[:, 0:8],
                                                srow[:], -3e38)
                        nc.vector.max(w16[:, 8:16], srow[:])
                        nc.vector.max_index(gi[:, 8:16], w16[:, 8:16],
                                            srow[:])
                        nc.vector.tensor_copy(gip[:, 0:16], gi[:])
                        for b_ in range(4):
                            nc.vector.transpose(
                                giT_all[0:32,
                                        bass.ds(t * P + 32 * b_, 32)],
                                gip[32 * b_:32 * b_ + 32, 0:32])

                # ---- replicate wrapped indices to all 8 gpsimd groups ----
                for g in range(8):
                    nc.sync.dma_start(wrap_all[16 * g:16 * g + 16, :],
                                      giT_all[0:16, :])

                # ---- block1 gather-max (2 query tiles per gather) ----
                with tc.tile_pool(name="g1pool", bufs=2) as g1pool:
                    with tc.For_i(0, nt // 2, 2) as t1:
                        for u in range(2):
                            tsl = bass.ds(t1 * 2 * P + u * 2 * P, 2 * P)
                            gat = g1pool.tile([P, 2 * P * K], F32,
                                              tag="gat", name="gat")
                            nc.gpsimd.ap_gather(
                                gat[:].rearrange("c (n d) -> c n d", d=1),
                                U1T[:].rearrange("c (n d) -> c n d", d=1),
                                wrap_all[:, tsl].bitcast(dt.int16),
                                channels=P, num_elems=n, d=1,
                                num_idxs=2 * P * K)
                            h1 = lpool.tile([P, 2 * P], F32, tag="h1",
                                            name="h1")
                            nc.vector.tensor_reduce(
                                h1[:],
                                gat[:].rearrange("c (q s) -> c q s", s=K),
                                axis=AX, op=MAX)
                            nc.vector.tensor_tensor(h1[:], h1[:],
                                                    f1T[:, tsl], op=ADD)
                            nc.scalar.activation(f1T[:, tsl], h1[:], Relu,
                                                 bias=b1_b, scale=1.0)

            # ---- exchange f1 halves within the pair ----
            nc.sync.dma_start(f1_loc.ap(), f1T[:])
            if PAIRS:
                nc.gpsimd.collective_compute(
                    "AllGather", mybir.AluOpType.bypass,
                    replica_groups=PAIRS,
                    ins=[f1_loc.ap()], outs=[f1_gath.ap()])
            else:   # single-core build (cost-model runs): fake the gather
                for r in range(HALVES):
                    nc.sync.dma_start(f1_gath.ap()[r], f1_loc.ap())

            with tc.tile_pool(name="poolD", bufs=1) as poolD:
                U2T = [poolD.tile([P, n], F32, tag=f"U2T{o}",
                                  name=f"U2T{o}") for o in range(2)]
                f2T = [poolD.tile([P, nq], F32, tag=f"f2T{o}",
                                  name=f"f2T{o}") for o in range(2)]
                for r in range(2):
                    with tc.For_i(0, nch // 2, 2) as ju:
                        for u in range(2):
                            qo = ju * CH + u * CH
                            sl = bass.ds(r * nq + qo, CH)
                            fch = stg.tile([W1, CH], F32, tag="fch",
                                           name="fch")
                            nc.sync.dma_start(
                                fch[:],
                                f1_gath.ap()[r, :, bass.ds(qo, CH)])
                            cch = stg.tile([3, CH], F32, tag="cch",
                                           name="cch")
                            nc.sync.dma_start(cch[:],
                                              coords_dram.ap()[:, sl])
                            for o in range(2):
                                osl = slice(o * P, o * P + P)
                                mm_chain(U2T[o], sl,
                                         [(W["w2_u_a"][:, osl], fch[:]),
                                          (W["w2_u_b"][:, osl], cch[:])])
                # V2 -> f2T storage
                with tc.For_i(0, nqch) as iv:
                    sl = bass.ts(iv, CH)
                    for o in range(2):
                        osl = slice(o * P, o * P + P)
                        mm_chain(f2T[o], sl,
                                 [(W["w2_v_a"][:, osl], f1T[:, sl]),
                                  (W["w2_v_b"][:, osl], q5[0:3, sl])])

                # ---- block2 gather-max -> f2 (in place) ----
                with tc.tile_pool(name="g2pool", bufs=1) as g2pool:
                    with tc.For_i(0, nt // 2) as t2:
                        tsl = bass.ts(t2, 2 * P)
                        for o in range(2):
                            gat = g2pool.tile([P, 2 * P * K], F32,
                                              tag="gat", name="gat")
                            nc.gpsimd.ap_gather(
                                gat[:].rearrange("c (n d) -> c n d", d=1),
                                U2T[o][:].rearrange("c (n d) -> c n d",
                                                    d=1),
                                wrap_all[:, tsl].bitcast(dt.int16),
                                channels=P, num_elems=n, d=1,
                                num_idxs=2 * P * K)
                            h2 = lpool.tile([P, 2 * P], F32, tag="h1")
                            nc.vector.tensor_reduce(
                                h2[:],
                                gat[:].rearrange("c (q s) -> c q s", s=K),
                                axis=AX, op=MAX)
                            nc.vector.tensor_tensor(h2[:], h2[:],
                                                    f2T[o][:, tsl],
                                                    op=ADD)
                            nc.scalar.activation(f2T[o][:, tsl], h2[:],
                                                 Relu,
                                                 bias=b2_b[:, o:o + 1],
                                                 scale=1.0)

                # ---- global max pool + glob MLP + b_eff ----
                gmx = pers.tile([P, 2], F32, tag="gmx")
                for o in range(2):
                    nc.vector.tensor_reduce(gmx[:, o:o + 1], f2T[o][:],
                                            axis=AX, op=MAX)
                    nc.sync.dma_start(g_loc.ap()[o * P:o * P + P, :],
                                      gmx[:, o:o + 1])
                if PAIRS:
                    nc.gpsimd.collective_compute(
                        "AllReduce", MAX, replica_groups=PAIRS,
                        ins=[g_loc.ap()], outs=[g_red.ap()])
                else:
                    nc.sync.dma_start(g_red.ap(), g_loc.ap())
                gsb = pers.tile([P, 2], F32, tag="gsb")
                nc.sync.dma_start(
                    gsb[:], g_red.ap().rearrange("(r p) c -> p (r c)", r=2))
                g2 = pers.tile([P, 2], F32, tag="g2")
                beff = pers.tile([P, 2], F32, tag="beff")
                for o in range(2):
                    osl = slice(o * P, o * P + P)
                    mm_chain(g2, slice(o, o + 1),
                             [(W["glob_k0"][:, osl], gsb[:, 0:1]),
                              (W["glob_k1"][:, osl], gsb[:, 1:2])],
                             act=Relu, bias=glob_b[:, o:o + 1],
                             shape=(P, 1))
                for o in range(2):
                    osl = slice(o * P, o * P + P)
                    ps = ppool.tile([P, 1], F32, tag="mm", name="beffps")
                    nc.tensor.matmul(ps[:], W["h1g_k0"][:, osl], g2[:, 0:1],
                                     start=True, stop=False)
                    nc.tensor.matmul(ps[:], W["h1g_k1"][:, osl], g2[:, 1:2],
                                     start=False, stop=True)
                    nc.vector.tensor_scalar(beff[:, o:o + 1], ps[:],
                                            h1_b[:, o:o + 1], None,
                                            op0=ADD)

                # ---- head (512-wide chunks) ----
                with tc.tile_pool(name="headpool", bufs=2) as headpool:
                    with tc.For_i(0, nqch) as ih:
                        sl = bass.ts(ih, CH)
                        hT = [headpool.tile([P, CH], F32, tag=f"hT{o}",
                                            name=f"hT{o}")
                              for o in range(2)]
                        for o in range(2):
                            osl = slice(o * P, o * P + P)
                            ps = ppool.tile([P, CH], F32, tag="mm",
                                            name="hps")
                            nc.tensor.matmul(ps[:], W["h1a_k0"][:, osl],
                                             f2T[0][:, sl],
                                             start=True, stop=False)
                            nc.tensor.matmul(ps[:], W["h1a_k1"][:, osl],
                                             f2T[1][:, sl],
                                             start=False, stop=True)
                            nc.scalar.activation(hT[o][:], ps[:], Relu,
                                                 bias=beff[:, o:o + 1],
                                                 scale=1.0)
                        ps3 = ppool.tile([NUM_CLASSES, CH], F32, tag="mm",
                                         name="lps")
                        nc.tensor.matmul(ps3[:], W["h2_k0"][:], hT[0][:],
                                         start=True, stop=False)
                        nc.tensor.matmul(ps3[:], W["h2_k1"][:], hT[1][:],
                                         start=False, stop=True)
                        lg = lpool.tile([NUM_CLASSES, CH], F32, tag="lg")
                        nc.vector.tensor_scalar(lg[:], ps3[:],
                                                h2b[:, 0:1], None, op0=ADD)
                        sg = lpool.tile([1, CH], F32, tag="sg")
                        hagt = lpool.tile([1, CH], F32, tag="hagt")
                        nc.sync.dma_start(hagt[:], qp.ap()[3:4, sl])
                        # sig: [-sharp, sharp*thresh, scale]
                        nc.scalar.activation(sg[:], hagt[:], Sigmoid,
                                             bias=sig[0:1, 1:2],
                                             scale=sig[0:1, 0:1])
                        nc.vector.scalar_tensor_tensor(
                            lg[0:1, :], sg[:], sig[0:1, 2:3],
                            lg[0:1, :], op0=mybir.AluOpType.mult, op1=ADD)
                        lg16 = lpool.tile([NUM_CLASSES, CH], F16,
                                          tag="lg16", name="lg16")
                        nc.scalar.activation(lg16[:], lg[:], Copy)
                        nc.sync.dma_start(out_lg.ap()[:, sl], lg16[:])

    nc.compile()
    return nc


def prep_inputs(x, hmix_a, hmix_b, hmix_c, stem_w, stem_b, b1_w, b1_b,
                b2_w, b2_b, glob_w, glob_b, head1_w, head1_b,
                head2_w, head2_b, thresh, sharp, scale, n=N, ncores=NCORES):
    """Host-side layout prep: per-core input maps (data movement + weight
    repacking only)."""
    f = np.float32
    nq = n // HALVES
    x = np.asarray(x, f)

    b1_w = np.asarray(b1_w, f); b2_w = np.asarray(b2_w, f)
    w1_f, w1_df, w1_dp = b1_w[0:W0], b1_w[W0:2 * W0], b1_w[2 * W0:]
    w2_f, w2_df, w2_dp = b2_w[0:W1], b2_w[W1:2 * W1], b2_w[2 * W1:]
    head1_w = np.asarray(head1_w, f)
    glob_w = np.asarray(glob_w, f); head2_w = np.asarray(head2_w, f)

    pack = np.zeros(WTOT, np.float16)
    wvals = {
        "w2_u_a": w2_df, "w1_u_a0": w1_df[0:32], "w2_v_a": w2_f - w2_df,
        "w1_u_a1": w1_df[32:64], "glob_k0": glob_w[0:128],
        "w1_v_a0": (w1_f - w1_df)[0:32], "glob_k1": glob_w[128:256],
        "w1_v_a1": (w1_f - w1_df)[32:64],
        "h1a_k0": head1_w[0:128], "w2_u_b": w2_dp, "w2_v_b": -w2_dp,
        "w1_u_b": w1_dp, "w1_v_b": -w1_dp,
        "stem_w": np.asarray(stem_w, f),
        "h2_k0": head2_w[0:128], "h2_k1": head2_w[128:256],
        "h1a_k1": head1_w[128:256], "h1g_k0": head1_w[256:384],
        "h1g_k1": head1_w[384:512],
    }
    for name, (off, p_, f_) in WOFF.items():
        v = np.asarray(wvals[name], f)
        assert v.shape == (p_, f_), (name, v.shape, (p_, f_))
        pack[off:off + p_ * f_] = v.astype(np.float16).ravel()
    pack = pack.reshape(NCORES, WSHARD)

    params = np.zeros(2048, f)
    Bt = np.zeros((P, 8), f)
    Bt[0:W0, 0] = np.asarray(stem_b, f)
    Bt[:, 1] = np.asarray(b1_b, f)
    Bt[:, 2:4] = np.asarray(b2_b, f).reshape(2, P).T
    Bt[:, 4:6] = np.asarray(glob_b, f).reshape(2, P).T
    Bt[:, 6:8] = np.asarray(head1_b, f).reshape(2, P).T
    params[PB_BT:PB_BT + 1024] = Bt.ravel()
    mc4 = np.zeros((4, 4), f)
    mc4[0, 0] = 1.0
    mc4[1, 1] = 1.0
    mc4[2, 2] = float(hmix_a)
    mc4[3, 2] = float(hmix_b)
    mc4[0:3, 3] = [0.0, 0.0, float(hmix_c)]
    params[PB_MC4:PB_MC4 + 16] = mc4.ravel()
    params[PB_H2B:PB_H2B + 3] = np.asarray(head2_b, f)
    params[PB_SIG:PB_SIG + 3] = [-float(sharp), float(sharp) * float(thresh),
                                 float(scale)]

    in_maps = []
    for c in range(ncores):
        b, h = c // HALVES, c % HALVES
        xT = np.ascontiguousarray(x[b % B].T[:, :n])
        xp = np.concatenate([xT, np.zeros((1, n), f)], 0)
        xp[4, 0:2048] = params
        qpv = np.ascontiguousarray(xT[:, h * nq:(h + 1) * nq])
        in_maps.append({"xp": xp, "qp": qpv,
                        "wsh": pack[c % NCORES:c % NCORES + 1]})
    return in_maps


_CACHE = {}


def kernel(**inputs):
    from concourse.bass_utils import run_bass_kernel_spmd
    if "nc" not in _CACHE:
        _CACHE["nc"] = build_program()
    nc = _CACHE["nc"]
    in_maps = prep_inputs(**inputs)
    r = run_bass_kernel_spmd(nc, in_maps, list(range(NCORES)))
    nq = N // HALVES
    out = np.zeros((B, N, NUM_CLASSES), np.float32)
    for c in range(NCORES):
        b, h = c // HALVES, c % HALVES
        out[b, h * nq:(h + 1) * nq, :] = \
            r.results[c]["out_lg"].T.astype(np.float32)
    return out


# revision 40
# speedup vs baseline: 1.2507x; 1.2507x over previous
"""HeightAwarePointNetTiny on 8 Trainium2 NeuronCores (Bass/Tile), v2.

Same math as v1 (see kernel.py docstring): each LocalAggBlock is
    out_i = relu(v_i + max_{j in KNN(i)} u_j)
with u/v from split weights, KNN top-16 via PE score rows + DVE
max8/max_index/match_replace, neighbor gathers via GPSIMD ap_gather.

v2 changes target the per-call host/tunnel overhead that dominates
wall-clock:
- A persistent XLA compilation cache (run_bass_kernel_spmd re-jits a
  fresh closure per call, which otherwise re-runs the full NEFF
  compile, ~0.6 s/call).
- The 27 per-core input tensors (~11 MB/call on the wire, weights
  replicated x8) are repacked into three:
    xp  f32 [5, 8192]  rows 0-3 = cloud x^T, row 4 = fp32 params
    qp  f32 [4, 4096]  this core's query half of x^T
    wsh f16 [1, 36864] 1/8th shard of the fp16 weight pack; the full
                       pack is reassembled on-device with an 8-way
                       AllGather, then upcast to fp32 in SBUF
  (~1.9 MB total; x stays fp32 — fp16 coords flip KNN neighbor sets
  and eat most of the 2e-2 error budget).  Logits return as fp16.
- The hot loops are hardware loops (tc.For_i), shrinking the program
  from ~3900 to ~1300 instructions: smaller BIR serialization, trace,
  and executable load on every call.  Caveats hit: matmul stationary
  operands forbid register offsets (queries staged via a copy), and a
  dynamic free-axis offset combined with a nonzero partition offset
  miscomputes (xx rows staged through DRAM instead).
- Per-tile wrap_all replication DMAs (8/tile) batched into 8 total;
  gathers take two query tiles each; the head runs 512 wide.

Sharding: core c owns cloud c//2, query half c%2 (4096 rows).  Cross-core
data: weight shards (AllGather over all 8), f1 (AllGather over pairs),
global max pool (AllReduce-max over pairs).
"""
import sys
sys.path.insert(0, '/opt/trn_rl_repo')
import numpy as np
from contextlib import ExitStack

try:
    # Persistent XLA compilation cache: run_bass_kernel_spmd re-jits a fresh
    # closure every call, so without this each kernel() invocation pays a
    # full NEFF recompile (~0.6 s) even though the program is unchanged.
    import jax
    jax.config.update("jax_compilation_cache_dir", "/tmp/jaxcache")
    jax.config.update("jax_persistent_cache_min_entry_size_bytes", 0)
    jax.config.update("jax_persistent_cache_min_compile_time_secs", 0.0)
except Exception:
    pass

import concourse.bass as bass
import concourse.tile as tile
from concourse import bacc, mybir

dt = mybir.dt
F32 = dt.float32
F16 = dt.float16

B, N, IN_CH = 4, 8192, 4
K = 16
W0, W1, W2 = 64, 128, 256
NUM_CLASSES = 3
NCORES = 8
P = 128
CH = 512
HALVES = 2
WSHARD = 36864                      # fp16 elems per core shard
WTOT = NCORES * WSHARD

# flat fp16 pack offsets (within the full 8*WSHARD pack)
_S = WSHARD
WOFF = {
    "w2_u_a": (0 * _S, 128, 256),
    "w1_u_a0": (0 * _S + 32768, 32, 128),
    "w2_v_a": (1 * _S, 128, 256),
    "w1_u_a1": (1 * _S + 32768, 32, 128),
    "glob_k0": (2 * _S, 128, 256),
    "w1_v_a0": (2 * _S + 32768, 32, 128),
    "glob_k1": (3 * _S, 128, 256),
    "w1_v_a1": (3 * _S + 32768, 32, 128),
    "h1a_k0": (4 * _S, 128, 256),
    "w2_u_b": (4 * _S + 32768, 3, 256),
    "w2_v_b": (4 * _S + 33536, 3, 256),
    "w1_u_b": (4 * _S + 34304, 3, 128),
    "w1_v_b": (4 * _S + 34688, 3, 128),
    "stem_w": (4 * _S + 35072, 4, 64),
    "h2_k0": (4 * _S + 35328, 128, 3),
    "h2_k1": (4 * _S + 35712, 128, 3),
    "h1a_k1": (5 * _S, 128, 256),
    "h1g_k0": (6 * _S, 128, 256),
    "h1g_k1": (7 * _S, 128, 256),
}
# fp32 params layout in xp row 4
PB_BT = 0           # Bt [128, 8]: cols stem_b(64), b1_b, b2_b0, b2_b1,
                    #              glob_b0, glob_b1, h1_b0, h1_b1
PB_MC4 = 1024       # mc4 [4, 4]: cols 0-2 height-mixer, col 3 rows 0-2 cvec
PB_H2B = 1040       # h2_b [3, 1]
PB_SIG = 1043       # sig_par [1, 3]


def build_program(n=N, ncores=NCORES):
    nq = n // HALVES
    nt = nq // P
    nch = n // CH
    nqch = nq // CH
    nc = bacc.Bacc("TRN2", target_bir_lowering=False, debug=False,
                   num_devices=ncores)

    xp = nc.dram_tensor("xp", [5, n], F32, kind="ExternalInput")
    qp = nc.dram_tensor("qp", [4, nq], F32, kind="ExternalInput")
    wsh = nc.dram_tensor("wsh", [1, WSHARD], F16, kind="ExternalInput")

    out_lg = nc.dram_tensor("out_lg", [NUM_CLASSES, nq], F16,
                            kind="ExternalOutput")
    wloc = nc.dram_tensor("wloc", [1, WSHARD], F16)
    wfull = nc.dram_tensor("wfull", [NCORES, WSHARD], F16)
    coords_dram = nc.dram_tensor("coords_dram", [3, n], F32)
    xx_dram = nc.dram_tensor("xx_dram", [1, n], F32)
    xxq_dram = nc.dram_tensor("xxq_dram", [1, nq], F32)
    f1_loc = nc.dram_tensor("f1_loc", [W1, nq], F32)
    f1_gath = nc.dram_tensor("f1_gath", [HALVES, W1, nq], F32)
    g_loc = nc.dram_tensor("g_loc", [W2, 1], F32)
    g_red = nc.dram_tensor("g_red", [W2, 1], F32)
    PAIRS = [[c, c + 1] for c in range(0, ncores, 2)] if ncores > 1 else []
    ALL8 = [list(range(ncores))] if ncores > 1 else []

    Relu = mybir.ActivationFunctionType.Relu
    Copy = mybir.ActivationFunctionType.Copy
    Sigmoid = mybir.ActivationFunctionType.Sigmoid
    Square = mybir.ActivationFunctionType.Square
    AX = mybir.AxisListType.X
    MAX = mybir.AluOpType.max
    ADD = mybir.AluOpType.add

    with tile.TileContext(nc) as tc, ExitStack() as ctx:
        pers = ctx.enter_context(tc.tile_pool(name="pers", bufs=1))
        lpool = ctx.enter_context(tc.tile_pool(name="lp", bufs=2))
        stg = ctx.enter_context(tc.tile_pool(name="stg", bufs=2))
        ppool = ctx.enter_context(tc.tile_pool(name="ps", bufs=4, space="PSUM"))

        def mm_chain(dst, dst_sl, parts, act=Copy, bias=0.0, scale=1.0,
                     shape=(P, CH)):
            ps = ppool.tile(list(shape), F32, tag="mm", name="mmps")
            for ix, (lhsT, rhs) in enumerate(parts):
                nc.tensor.matmul(ps[:], lhsT, rhs, start=(ix == 0),
                                 stop=(ix == len(parts) - 1))
            nc.scalar.activation(dst[:, dst_sl], ps[:], act, bias=bias,
                                 scale=scale)

        # ---- weight shard exchange ----
        nc.sync.dma_start(wloc.ap(), wsh.ap())
        if ALL8:
            nc.gpsimd.collective_compute(
                "AllGather", mybir.AluOpType.bypass, replica_groups=ALL8,
                ins=[wloc.ap()], outs=[wfull.ap()])
        else:
            for r in range(NCORES):
                nc.sync.dma_start(wfull.ap()[r], wloc.ap())

        # ---- unpack fp32 params from xp row 4 ----
        def par_tile(tag, p_, f_, off):
            t32 = pers.tile([p_, f_], F32, tag=tag, name=tag)
            nc.sync.dma_start(
                t32[:], xp.ap()[4, off:off + p_ * f_].rearrange(
                    "(p f) -> p f", p=p_))
            return t32

        Bt = par_tile("Bt", P, 8, PB_BT)
        mc4 = par_tile("mc4", 4, 4, PB_MC4)
        h2b = par_tile("h2b", NUM_CLASSES, 1, PB_H2B)
        sig = par_tile("sig", 1, 3, PB_SIG)

        # ---- unpack fp16 weights -> fp32 SBUF tiles ----
        W = {}
        for name, (off, p_, f_) in WOFF.items():
            if name[-1] in "01" and name[:-1] in ("w1_u_a", "w1_v_a"):
                continue
            s_, o_ = off // WSHARD, off % WSHARD
            t16 = stg.tile([p_, f_], F16, tag="w16stg", name=f"s_{name}")
            nc.sync.dma_start(
                t16[:], wfull.ap()[s_, o_:o_ + p_ * f_].rearrange(
                    "(p f) -> p f", p=p_))
            t32 = pers.tile([p_, f_], F32, tag=f"W_{name}", name=name)
            nc.scalar.activation(t32[:], t16[:], Copy)
            W[name] = t32
        # split [32, W1] halves: DMA both into one fp16 tile, convert once
        for base in ("w1_u_a", "w1_v_a"):
            t16 = stg.tile([W0, W1], F16, tag="w16stg", name=f"s_{base}")
            for half in (0, 1):
                off, p_, f_ = WOFF[f"{base}{half}"]
                s_, o_ = off // WSHARD, off % WSHARD
                nc.sync.dma_start(
                    t16[32 * half:32 * half + 32, :],
                    wfull.ap()[s_, o_:o_ + p_ * f_].rearrange(
                        "(p f) -> p f", p=p_))
            t32 = pers.tile([W0, W1], F32, tag=f"W_{base}", name=base)
            nc.scalar.activation(t32[:], t16[:], Copy)
            W[base] = t32
        w1_u_a = W["w1_u_a"]
        w1_v_a = W["w1_v_a"]

        stem_b = Bt[0:W0, 0:1]
        b1_b = Bt[:, 1:2]
        b2_b = Bt[:, 2:4]
        glob_b = Bt[:, 4:6]
        h1_b = Bt[:, 6:8]
        cvec = mc4[0:3, 3:4]

        wrap_all = pers.tile([P, nt * P], dt.uint16, tag="wrap_all")
        ones3 = pers.tile([3, 1], F32, tag="ones3")
        nc.vector.memset(ones3[:], 1.0)

        with tc.tile_pool(name="poolC", bufs=1) as poolC:
            q5 = poolC.tile([5, nq], F32, tag="q5")
            f1T = poolC.tile([W1, nq], F32, tag="f1T")


            with tc.tile_pool(name="poolB", bufs=1) as poolB:
                rhs5 = poolB.tile([5, n], F32, tag="rhs5")
                U1T = poolB.tile([W1, n], F32, tag="U1T")
                giT_all = poolB.tile([32, nt * P], dt.uint16, tag="giT_all")
                # rows 0-2 and 4 are overwritten below; row 3 keeps -1
                nc.vector.memset(rhs5[:], -1.0)
                # rows 0-3 are overwritten below; row 4 keeps +1
                nc.vector.memset(q5[:], 1.0)

                # ---- streamed setup over candidate chunks ----
                def cand_chunk(sl, src):
                    xch = stg.tile([4, CH], F32, tag="xch", name="xch")
                    nc.sync.dma_start(xch[:], src)
                    cch = stg.tile([3, CH], F32, tag="cch", name="cch")
                    ps = ppool.tile([3, CH], F32, tag="mm", name="csps")
                    nc.tensor.matmul(ps[:], mc4[0:4, 0:3], xch[:],
                                     start=True, stop=True)
                    nc.vector.tensor_scalar(cch[:], ps[:], cvec, None,
                                            op0=ADD)
                    nc.scalar.activation(rhs5[0:3, sl], cch[:], Copy,
                                         scale=2.0)
                    nc.sync.dma_start(coords_dram.ap()[:, sl], cch[:])
                    sqs = stg.tile([3, CH], F32, tag="sqs", name="sqs")
                    nc.scalar.activation(sqs[:], cch[:], Square)
                    psx = ppool.tile([1, CH], F32, tag="mm", name="xxps")
                    nc.tensor.matmul(psx[:], ones3[:], sqs[:],
                                     start=True, stop=True)
                    xxs = stg.tile([1, CH], F32, tag="xxs", name="xxs")
                    nc.scalar.activation(xxs[:], psx[:], Copy, scale=-1.0)
                    nc.sync.dma_start(xx_dram.ap()[:, sl], xxs[:])
                    f64 = stg.tile([W0, CH], F32, tag="f64", name="f64")
                    mm_chain(f64, slice(0, CH),
                             [(W["stem_w"][:], xch[:])],
                             act=Relu, bias=stem_b, shape=(W0, CH))
                    mm_chain(U1T, sl, [(w1_u_a[:], f64[:]),
                                       (W["w1_u_b"][:], cch[:])])

                with tc.For_i(0, nch, 2) as ic:
                    for u in range(2):
                        sl = bass.ds(ic * CH + u * CH, CH)
                        cand_chunk(sl, xp.ap()[0:4, sl])
                nc.sync.dma_start(rhs5[4:5, :], xx_dram.ap())

                # ---- streamed setup over query chunks (V1 -> f1T) ----
                def query_chunk(sl):
                    xch = stg.tile([4, CH], F32, tag="xch", name="xch")
                    nc.sync.dma_start(xch[:], qp.ap()[:, sl])
                    ps = ppool.tile([3, CH], F32, tag="mm", name="qcps")
                    nc.tensor.matmul(ps[:], mc4[0:4, 0:3], xch[:],
                                     start=True, stop=True)
                    nc.vector.tensor_scalar(q5[0:3, sl], ps[:], cvec, None,
                                            op0=ADD)
                    sqs = stg.tile([3, CH], F32, tag="sqs", name="sqs")
                    nc.scalar.activation(sqs[:], q5[0:3, sl], Square)
                    psx = ppool.tile([1, CH], F32, tag="mm", name="xxpsq")
                    nc.tensor.matmul(psx[:], ones3[:], sqs[:],
                                     start=True, stop=True)
                    xxs = stg.tile([1, CH], F32, tag="xxs", name="xxs")
                    nc.scalar.activation(xxs[:], psx[:], Copy)
                    nc.sync.dma_start(xxq_dram.ap()[:, sl], xxs[:])
                    f64 = stg.tile([W0, CH], F32, tag="f64", name="f64")
                    mm_chain(f64, slice(0, CH),
                             [(W["stem_w"][:], xch[:])],
                             act=Relu, bias=stem_b, shape=(W0, CH))
                    mm_chain(f1T, sl, [(w1_v_a[:], f64[:]),
                                       (W["w1_v_b"][:], q5[0:3, sl])])

                with tc.For_i(0, nqch, 2) as iq:
                    for u in range(2):
                        query_chunk(bass.ds(iq * CH + u * CH, CH))
                nc.sync.dma_start(q5[3:4, :], xxq_dram.ap())

                # ---- selection: srow -> exact top-16 -> giT_all ----
                with tc.tile_pool(name="spool", bufs=1) as spool:
                    srow = spool.tile([P, n], F32, tag="srow", name="srow")
                    w16 = lpool.tile([P, K], F32, tag="w16", name="w16")
                    gi = lpool.tile([P, K], dt.uint16, tag="gi", name="gi")
                    gip = lpool.tile([P, 32], dt.uint16, tag="gip",
                                     name="gip")
                    qt5 = spool.tile([5, P], F32, tag="qt5", name="qt5")
                    nc.vector.memset(gip[:], 0)
                    with tc.For_i(0, nt) as t:
                        tsl = bass.ts(t, P)
                        nc.scalar.activation(qt5[:], q5[:, tsl], Copy)
                        for i in range(nch):
                            ps = ppool.tile([P, CH], F32, tag="mm",
                                            name="sps")
                            nc.tensor.matmul(ps[:], qt5[:],
                                             rhs5[:, bass.ts(i, CH)],
                                             start=True, stop=True)
                            nc.scalar.activation(srow[:, bass.ts(i, CH)],
                                                 ps[:], Copy, scale=1.0)
                        nc.vector.max(w16[:, 0:8], srow[:])
                        nc.vector.max_index(gi[:, 0:8], w16[:, 0:8],
                                            srow[:])
                        nc.vector.match_replace(srow[:], w16[:, 0:8],
                                                srow[:], -3e38)
                        nc.vector.max(w16[:, 8:16], srow[:])
                        nc.vector.max_index(gi[:, 8:16], w16[:, 8:16],
                                            srow[:])
                        nc.vector.tensor_copy(gip[:, 0:16], gi[:])
                        for b_ in range(4):
                            nc.vector.transpose(
                                giT_all[0:32,
                                        bass.ds(t * P + 32 * b_, 32)],
                                gip[32 * b_:32 * b_ + 32, 0:32])

                # ---- replicate wrapped indices to all 8 gpsimd groups ----
                for g in range(8):
                    nc.sync.dma_start(wrap_all[16 * g:16 * g + 16, :],
                                      giT_all[0:16, :])

                # ---- block1 gather-max (2 query tiles per gather) ----
                with tc.tile_pool(name="g1pool", bufs=2) as g1pool:
                    with tc.For_i(0, nt // 2, 2) as t1:
                        for u in range(2):
                            tsl = bass.ds(t1 * 2 * P + u * 2 * P, 2 * P)
                            gat = g1pool.tile([P, 2 * P * K], F32,
                                              tag="gat", name="gat")
                            nc.gpsimd.ap_gather(
                                gat[:].rearrange("c (n d) -> c n d", d=1),
                                U1T[:].rearrange("c (n d) -> c n d", d=1),
                                wrap_all[:, tsl].bitcast(dt.int16),
                                channels=P, num_elems=n, d=1,
                                num_idxs=2 * P * K)
                            h1 = lpool.tile([P, 2 * P], F32, tag="h1",
                                            name="h1")
                            nc.vector.tensor_reduce(
                                h1[:],
                                gat[:].rearrange("c (q s) -> c q s", s=K),
                                axis=AX, op=MAX)
                            nc.vector.tensor_tensor(h1[:], h1[:],
                                                    f1T[:, tsl], op=ADD)
                            nc.scalar.activation(f1T[:, tsl], h1[:], Relu,
                                                 bias=b1_b, scale=1.0)

            # ---- exchange f1 halves within the pair ----
            nc.sync.dma_start(f1_loc.ap(), f1T[:])
            if PAIRS:
                nc.gpsimd.collective_compute(
                    "AllGather", mybir.AluOpType.bypass,
                    replica_groups=PAIRS,
                    ins=[f1_loc.ap()], outs=[f1_gath.ap()])
            else:   # single-core build (cost-model runs): fake the gather
                for r in range(HALVES):
                    nc.sync.dma_start(f1_gath.ap()[r], f1_loc.ap())

            with tc.tile_pool(name="poolD", bufs=1) as poolD:
                U2T = [poolD.tile([P, n], F32, tag=f"U2T{o}",
                                  name=f"U2T{o}") for o in range(2)]
                f2T = [poolD.tile([P, nq], F32, tag=f"f2T{o}",
                                  name=f"f2T{o}") for o in range(2)]
                for r in range(2):
                    with tc.For_i(0, nch // 2, 2) as ju:
                        for u in range(2):
                            qo = ju * CH + u * CH
                            sl = bass.ds(r * nq + qo, CH)
                            fch = stg.tile([W1, CH], F32, tag="fch",
                                           name="fch")
                            nc.sync.dma_start(
                                fch[:],
                                f1_gath.ap()[r, :, bass.ds(qo, CH)])
                            cch = stg.tile([3, CH], F32, tag="cch",
                                           name="cch")
                            nc.sync.dma_start(cch[:],
                                              coords_dram.ap()[:, sl])
                            for o in range(2):
                                osl = slice(o * P, o * P + P)
                                mm_chain(U2T[o], sl,
                                         [(W["w2_u_a"][:, osl], fch[:]),
                                          (W["w2_u_b"][:, osl], cch[:])])
                # V2 -> f2T storage
                with tc.For_i(0, nqch) as iv:
                    sl = bass.ts(iv, CH)
                    for o in range(2):
                        osl = slice(o * P, o * P + P)
                        mm_chain(f2T[o], sl,
                                 [(W["w2_v_a"][:, osl], f1T[:, sl]),
                                  (W["w2_v_b"][:, osl], q5[0:3, sl])])

                # ---- block2 gather-max -> f2 (in place) ----
                with tc.tile_pool(name="g2pool", bufs=1) as g2pool:
                    with tc.For_i(0, nt // 2) as t2:
                        tsl = bass.ts(t2, 2 * P)
                        for o in range(2):
                            gat = g2pool.tile([P, 2 * P * K], F32,
                                              tag="gat", name="gat")
                            nc.gpsimd.ap_gather(
                                gat[:].rearrange("c (n d) -> c n d", d=1),
                                U2T[o][:].rearrange("c (n d) -> c n d",
                                                    d=1),
                                wrap_all[:, tsl].bitcast(dt.int16),
                                channels=P, num_elems=n, d=1,
                                num_idxs=2 * P * K)
                            h2 = lpool.tile([P, 2 * P], F32, tag="h1")
                            nc.vector.tensor_reduce(
                                h2[:],
                                gat[:].rearrange("c (q s) -> c q s", s=K),
                                axis=AX, op=MAX)
                            nc.vector.tensor_tensor(h2[:], h2[:],
                                                    f2T[o][:, tsl],
                                                    op=ADD)
                            nc.scalar.activation(f2T[o][:, tsl], h2[:],
                                                 Relu,
                                                 bias=b2_b[:, o:o + 1],
                                                 scale=1.0)

                # ---- global max pool + glob MLP + b_eff ----
                gmx = pers.tile([P, 2], F32, tag="gmx")
                for o in range(2):
                    nc.vector.tensor_reduce(gmx[:, o:o + 1], f2T[o][:],
                                            axis=AX, op=MAX)
                    nc.sync.dma_start(g_loc.ap()[o * P:o * P + P, :],
                                      gmx[:, o:o + 1])
                if PAIRS:
                    nc.gpsimd.collective_compute(
                        "AllReduce", MAX, replica_groups=PAIRS,
                        ins=[g_loc.ap()], outs=[g_red.ap()])
                else:
                    nc.sync.dma_start(g_red.ap(), g_loc.ap())
                gsb = pers.tile([P, 2], F32, tag="gsb")
                nc.sync.dma_start(
                    gsb[:], g_red.ap().rearrange("(r p) c -> p (r c)", r=2))
                g2 = pers.tile([P, 2], F32, tag="g2")
                beff = pers.tile([P, 2], F32, tag="beff")
                for o in range(2):
                    osl = slice(o * P, o * P + P)
                    mm_chain(g2, slice(o, o + 1),
                             [(W["glob_k0"][:, osl], gsb[:, 0:1]),
                              (W["glob_k1"][:, osl], gsb[:, 1:2])],
                             act=Relu, bias=glob_b[:, o:o + 1],
                             shape=(P, 1))
                for o in range(2):
                    osl = slice(o * P, o * P + P)
                    ps = ppool.tile([P, 1], F32, tag="mm", name="beffps")
                    nc.tensor.matmul(ps[:], W["h1g_k0"][:, osl], g2[:, 0:1],
                                     start=True, stop=False)
                    nc.tensor.matmul(ps[:], W["h1g_k1"][:, osl], g2[:, 1:2],
                                     start=False, stop=True)
                    nc.vector.tensor_scalar(beff[:, o:o + 1], ps[:],
                                            h1_b[:, o:o + 1], None,
                                            op0=ADD)

                # ---- head (512-wide chunks) ----
                with tc.tile_pool(name="headpool", bufs=2) as headpool:
                    with tc.For_i(0, nqch) as ih:
                        sl = bass.ts(ih, CH)
                        hT = [headpool.tile([P, CH], F32, tag=f"hT{o}",
                                            name=f"hT{o}")
                              for o in range(2)]
                        for o in range(2):
                            osl = slice(o * P, o * P + P)
                            ps = ppool.tile([P, CH], F32, tag="mm",
                                            name="hps")
                            nc.tensor.matmul(ps[:], W["h1a_k0"][:, osl],
                                             f2T[0][:, sl],
                                             start=True, stop=False)
                            nc.tensor.matmul(ps[:], W["h1a_k1"][:, osl],
                                             f2T[1][:, sl],
                                             start=False, stop=True)
                            nc.scalar.activation(hT[o][:], ps[:], Relu,
                                                 bias=beff[:, o:o + 1],
                                                 scale=1.0)
                        ps3 = ppool.tile([NUM_CLASSES, CH], F32, tag="mm",
                                         name="lps")
                        nc.tensor.matmul(ps3[:], W["h2_k0"][:], hT[0][:],
                                         start=True, stop=False)
                        nc.tensor.matmul(ps3[:], W["h2_k1"][:], hT[1][:],
                                         start=False, stop=True)
                        lg = lpool.tile([NUM_CLASSES, CH], F32, tag="lg")
                        nc.vector.tensor_scalar(lg[:], ps3[:],
                                                h2b[:, 0:1], None, op0=ADD)
                        sg = lpool.tile([1, CH], F32, tag="sg")
                        hagt = lpool.tile([1, CH], F32, tag="hagt")
                        nc.sync.dma_start(hagt[:], qp.ap()[3:4, sl])
                        # sig: [-sharp, sharp*thresh, scale]
                        nc.scalar.activation(sg[:], hagt[:], Sigmoid,
                                             bias=sig[0:1, 1:2],
                                             scale=sig[0:1, 0:1])
                        nc.vector.scalar_tensor_tensor(
                            lg[0:1, :], sg[:], sig[0:1, 2:3],
                            lg[0:1, :], op0=mybir.AluOpType.mult, op1=ADD)
                        lg16 = lpool.tile([NUM_CLASSES, CH], F16,
                                          tag="lg16", name="lg16")
                        nc.scalar.activation(lg16[:], lg[:], Copy)
                        nc.sync.dma_start(out_lg.ap()[:, sl], lg16[:])

    nc.compile()
    return nc


def prep_inputs(x, hmix_a, hmix_b, hmix_c, stem_w, stem_b, b1_w, b1_b,
                b2_w, b2_b, glob_w, glob_b, head1_w, head1_b,
                head2_w, head2_b, thresh, sharp, scale, n=N, ncores=NCORES):
    """Host-side layout prep: per-core input maps (data movement + weight
    repacking only)."""
    f = np.float32
    nq = n // HALVES
    x = np.asarray(x, f)

    b1_w = np.asarray(b1_w, f); b2_w = np.asarray(b2_w, f)
    w1_f, w1_df, w1_dp = b1_w[0:W0], b1_w[W0:2 * W0], b1_w[2 * W0:]
    w2_f, w2_df, w2_dp = b2_w[0:W1], b2_w[W1:2 * W1], b2_w[2 * W1:]
    head1_w = np.asarray(head1_w, f)
    glob_w = np.asarray(glob_w, f); head2_w = np.asarray(head2_w, f)

    pack = np.zeros(WTOT, np.float16)
    wvals = {
        "w2_u_a": w2_df, "w1_u_a0": w1_df[0:32], "w2_v_a": w2_f - w2_df,
        "w1_u_a1": w1_df[32:64], "glob_k0": glob_w[0:128],
        "w1_v_a0": (w1_f - w1_df)[0:32], "glob_k1": glob_w[128:256],
        "w1_v_a1": (w1_f - w1_df)[32:64],
        "h1a_k0": head1_w[0:128], "w2_u_b": w2_dp, "w2_v_b": -w2_dp,
        "w1_u_b": w1_dp, "w1_v_b": -w1_dp,
        "stem_w": np.asarray(stem_w, f),
        "h2_k0": head2_w[0:128], "h2_k1": head2_w[128:256],
        "h1a_k1": head1_w[128:256], "h1g_k0": head1_w[256:384],
        "h1g_k1": head1_w[384:512],
    }
    for name, (off, p_, f_) in WOFF.items():
        v = np.asarray(wvals[name], f)
        assert v.shape == (p_, f_), (name, v.shape, (p_, f_))
        pack[off:off + p_ * f_] = v.astype(np.float16).ravel()
    pack = pack.reshape(NCORES, WSHARD)

    params = np.zeros(2048, f)
    Bt = np.zeros((P, 8), f)
    Bt[0:W0, 0] = np.asarray(stem_b, f)
    Bt[:, 1] = np.asarray(b1_b, f)
    Bt[:, 2:4] = np.asarray(b2_b, f).reshape(2, P).T
    Bt[:, 4:6] = np.asarray(glob_b, f).reshape(2, P).T
    Bt[:, 6:8] = np.asarray(head1_b, f).reshape(2, P).T
    params[PB_BT:PB_BT + 1024] = Bt.ravel()
    mc4 = np.zeros((4, 4), f)
    mc4[0, 0] = 1.0
    mc4[1, 1] = 1.0
    mc4[2, 2] = float(hmix_a)
    mc4[3, 2] = float(hmix_b)
    mc4[0:3, 3] = [0.0, 0.0, float(hmix_c)]
    params[PB_MC4:PB_MC4 + 16] = mc4.ravel()
    params[PB_H2B:PB_H2B + 3] = np.asarray(head2_b, f)
    params[PB_SIG:PB_SIG + 3] = [-float(sharp), float(sharp) * float(thresh),
                                 float(scale)]

    in_maps = []
    for c in range(ncores):
        b, h = c // HALVES, c % HALVES
        xT = np.ascontiguousarray(x[b % B].T[:, :n])
        xp = np.concatenate([xT, np.zeros((1, n), f)], 0)
        xp[4, 0:2048] = params
        qpv = np.ascontiguousarray(xT[:, h * nq:(h + 1) * nq])
        in_maps.append({"xp": xp, "qp": qpv,
                        "wsh": pack[c % NCORES:c % NCORES + 1]})
    return in_maps


_CACHE = {}


def kernel(**inputs):
    from concourse.bass_utils import run_bass_kernel_spmd
    if "nc" not in _CACHE:
        _CACHE["nc"] = build_program()
    nc = _CACHE["nc"]
    in_maps = prep_inputs(**inputs)
    r = run_bass_kernel_spmd(nc, in_maps, list(range(NCORES)))
    nq = N // HALVES
    out = np.zeros((B, N, NUM_CLASSES), np.float32)
    for c in range(NCORES):
        b, h = c // HALVES, c % HALVES
        out[b, h * nq:(h + 1) * nq, :] = \
            r.results[c]["out_lg"].T.astype(np.float32)
    return out
